# revision 45
# baseline (speedup 1.0000x reference)
"""MultiOutSizeLinear (MoE-style routed linear) for Trainium2, 8 NeuronCores.

Each token selects one of 4 experts by its ``out_feat_size`` value
(128/256/512/1024). Expert k is a dense [out_k, 1024] linear + bias whose
output lands in the first out_k columns of the 1024-wide output row; the
reference leaves bias[k, out_k:] in the remaining columns (zero for the
shipped setup_inputs, which pre-zeroes the bias tail).

Strategy
  host:   route tokens to experts; balance each expert's tokens evenly
          across the 8 cores (capacities are shared so one SPMD program
          serves all cores); gather + transpose each core's tokens into
          x^T [1024, TPAD] laid out as expert segments [e3 | e2 | e1 | e0],
          cast to bfloat16 (fp32 PSUM accumulation keeps rel err ~1e-3).
  device: keep W^T [1024, 1920] (all experts, concatenated out-columns), a
          128-row broadcast bias, the first 1024-token group of x^T, AND
          per-expert whole-segment output buffers resident in SBUF. ALL
          streaming DMAs go on the single SP HWDGE ring: 8 grouped 2 MB
          x^T reads (self-paced by buffer reuse) followed by just 4
          segment-sized bf16 output writes. Measured on HW: interleaved
          reads+writes thrash HBM read/write turnaround (in||out on two
          rings is 1.8x slower than serial), and each out-DMA's sem wait
          can stall the FIFO ring, so writes are made big and rare.
          All experts run token-stationary: psum[128 tok, out_k] +=
          xT_tile.T @ wT_tile, 8 accumulating K-tiles per <=512-wide
          column chunk (bf16 streams 1 col/cycle at any width, so expert 0
          needs no weight-stationary special case). Bias is added on
          VectorE during PSUM eviction into the segment buffers.
  host:   scatter rows back through the routing permutation (upcast to f32).
"""

import sys
import numpy as np

sys.path.insert(0, "/opt/trn_rl_repo")

OUT_SIZES = (128, 256, 512, 1024)
N_EXPERTS = len(OUT_SIZES)
IN_FEAT = 1024
N_CORES = 8
K_TILES = IN_FEAT // 128
BLK = 512       # tokens per x^T DRAM block
XGRP = 1        # 512-token blocks per x dma_start
CHUNK = BLK     # kept name: host DRAM layout is BLK-blocked
WOFF = tuple(int(np.cumsum((0,) + OUT_SIZES)[k]) for k in range(N_EXPERTS))
W_COLS = sum(OUT_SIZES)

_nc_cache: dict = {}


def _build(caps, repeat=1, loop=None, xbufs=3, obufs=None,
           drop_out=False, fake_x=False, resident_groups="auto",
           read_lead=None):
    """Compile the SPMD program for shared per-expert capacities ``caps``.

    caps[k] % 128 == 0; computed tokens sum(caps) need not be 512-aligned
    (the final DRAM block is padded). ``repeat``/``loop`` re-run the
    compute body (same I/O) for timing.
    """
    import concourse.bacc as bacc
    import concourse.mybir as mybir
    import concourse.tile as tile

    f32 = mybir.dt.float32
    bf16 = mybir.dt.bfloat16
    tpad = sum(caps)
    assert all(c % 128 == 0 for c in caps)
    nblocks = -(-tpad // BLK)          # DRAM blocks (last may be partial)
    ngroups = -(-nblocks // XGRP)

    if resident_groups == "auto":
        # resident x^T groups: whatever SBUF has left after the weights,
        # bias, and per-expert half-segment output buffers. When the whole
        # x^T fits (the common case), the body has ZERO read DMAs and the
        # streaming rotation pool is never used.
        grp_b = XGRP * K_TILES * BLK * 2
        fixed = (K_TILES * W_COLS * 2 + W_COLS * 2
                 + sum(-(-(caps[k] // 128) // 2) * OUT_SIZES[k] * 2
                       for k in range(N_EXPERTS)))
        if fixed + ngroups * grp_b <= 206 * 1024:
            resident_groups = ngroups
        else:
            resident_groups = max(0, (200 * 1024 - fixed - xbufs * grp_b)
                                  // grp_b)

    nc = bacc.Bacc(None, target_bir_lowering=False, debug=False)
    # block-layout x^T: block c holds tokens [c*BLK, (c+1)*BLK) as a
    # contiguous [IN_FEAT, BLK] slab; one dma_start covers XGRP blocks
    xt = nc.dram_tensor("xt", [nblocks, IN_FEAT, BLK], bf16,
                        kind="ExternalInput")
    wt = nc.dram_tensor("wt", [IN_FEAT, W_COLS], bf16, kind="ExternalInput")
    bb = nc.dram_tensor("bb", [128, W_COLS], bf16, kind="ExternalInput")
    # replicated bias for packed evictions: cols [0,512) = bias0 x4,
    # cols [512,1024) = bias1 x2
    bbr = nc.dram_tensor("bbr", [128, 1024], bf16, kind="ExternalInput")
    outs = {k: nc.dram_tensor(f"out{k}", [caps[k], OUT_SIZES[k]], bf16,
                              kind="ExternalOutput")
            for k in range(N_EXPERTS) if caps[k]}

    seg_order = [k for k in (3, 2, 1, 0) if caps[k] > 0]
    seg_start = {}
    t0 = 0
    for k in seg_order:
        seg_start[k] = t0
        t0 += caps[k]

    def expert_of(tok):
        for k in seg_order:
            if tok < seg_start[k] + caps[k]:
                return k
        raise AssertionError

    with tile.TileContext(nc) as tc:
        with (
            tc.tile_pool(name="const", bufs=1) as const,
            tc.tile_pool(name="xp", bufs=xbufs) as xp,
            tc.tile_pool(name="ps", bufs=4, space="PSUM") as psp,
        ):
            wt_sb = const.tile([128, K_TILES, W_COLS], bf16)
            nc.sync.dma_start(wt_sb[:], wt.rearrange("(kk p) n -> p kk n", p=128))
            bb_sb = const.tile([128, W_COLS], bf16)
            nc.sync.dma_start(bb_sb[:], bb[:])
            bbr_sb = const.tile([128, 1024], bf16)
            nc.sync.dma_start(bbr_sb[:], bbr[:])
            bbr_off = {0: 0, 1: 512}

            # half-segment output buffers: DVE evicts into slot bi % half;
            # flushed as ONE DMA per half per expert per iteration (the
            # 4-deep PSUM rotation absorbs the WAR wait when the second
            # half starts while the first half's flush drains)
            o_seg = {}
            o_half = {}
            for k in seg_order:
                nblk_k = caps[k] // 128
                o_half[k] = -(-nblk_k // 2)
                o_seg[k] = const.tile([128, o_half[k], OUT_SIZES[k]],
                                      bf16, name=f"oseg{k}")

            # blocks packed per PSUM tile (ok*pack <= 512): batches expert
            # 0/1 evictions so the PE->DVE->PE psum-recycle sem chain fires
            # 4x/2x less often in the fast small-expert tail
            packs = {}
            for k in seg_order:
                nblk_k = caps[k] // 128
                p = 1
                if k in bbr_off:
                    for cand in (4, 2):
                        if (cand * OUT_SIZES[k] <= 512
                                and nblk_k % cand == 0
                                and o_half[k] % cand == 0):
                            p = cand
                            break
                packs[k] = p

            def xsrc(gi):
                """(blocks, tokens-in-last-block, src AP) for x group gi.
                The final DRAM block is only read up to the last computed
                token."""
                s = gi * XGRP
                g = min(XGRP, nblocks - s)
                tk = BLK
                if s + g == nblocks and tpad % BLK and g == 1:
                    tk = tpad % BLK
                    src = xt[s:s + 1, :, :tk].rearrange(
                        "c (kk p) t -> p c kk t", p=128)
                else:
                    src = xt[s:s + g].rearrange("c (kk p) t -> p c kk t", p=128)
                return g, tk, src

            # leading groups of x^T stay resident: each loop iteration
            # starts computing on them immediately (loaded once, before the
            # loop), and they are not re-read per iteration
            nres = min(resident_groups, ngroups)
            xres = {}
            for gi in range(nres):
                gsz, tk, src = xsrc(gi)
                xr = const.tile([128, XGRP, K_TILES, BLK], bf16,
                                name=f"xres{gi}")
                nc.sync.dma_start(xr[:, :gsz, :, :tk], src)
                xres[gi] = xr

            def body():
                xtiles = dict(xres)

                def issue_x(gi):
                    if gi < nres or gi >= ngroups or fake_x:
                        return
                    g, tk, src = xsrc(gi)
                    x_sb = xp.tile([128, XGRP, K_TILES, BLK], bf16, tag="x")
                    nc.sync.dma_start(x_sb[:, :g, :, :tk], src)
                    xtiles[gi] = x_sb

                if read_lead is None:
                    for gi in range(nres, ngroups):
                        issue_x(gi)
                else:
                    for gi in range(nres, min(nres + read_lead, ngroups)):
                        issue_x(gi)

                next_read = nres + (read_lead or 0)
                cur_ps = None
                for tok in range(0, tpad, 128):
                    k = expert_of(tok)
                    ok = OUT_SIZES[k]
                    P = packs[k]
                    blk, off = divmod(tok, BLK)
                    gi, ci = divmod(blk, XGRP)
                    if read_lead is not None and off == 0 and ci == 0:
                        # compute cursor entered group gi: top up the lead
                        while next_read <= gi + read_lead and next_read < ngroups:
                            issue_x(next_read)
                            next_read += 1
                    x_sb = xtiles[0 if fake_x else gi]
                    bi = (tok - seg_start[k]) // 128
                    slot = bi % P
                    if slot == 0:
                        cur_ps = psp.tile([128, 1024], f32, tag="ps")
                    ps = cur_ps
                    for j0 in range(0, ok, 512):
                        jn = min(512, ok - j0)
                        for kk in range(K_TILES):
                            nc.tensor.matmul(
                                ps[:, slot * ok + j0:slot * ok + j0 + jn],
                                x_sb[:, ci, kk, off:off + 128],
                                wt_sb[:, kk, WOFF[k] + j0:WOFF[k] + j0 + jn],
                                start=(kk == 0), stop=(kk == K_TILES - 1))
                    if slot != P - 1:
                        continue
                    h = o_half[k]
                    b0 = bi - slot
                    bias = (bbr_sb[:, bbr_off[k]:bbr_off[k] + P * ok]
                            if P > 1 else bb_sb[:, WOFF[k]:WOFF[k] + ok])
                    nc.vector.tensor_add(
                        o_seg[k][:, b0 % h:b0 % h + P, :]
                        .rearrange("p j n -> p (j n)"),
                        ps[:, :P * ok], bias)
                    if drop_out:
                        continue
                    nblk_k = caps[k] // 128
                    if bi == h - 1 and h < nblk_k:
                        nc.sync.dma_start(
                            outs[k][:h * 128]
                            .rearrange("(j p) n -> p j n", p=128),
                            o_seg[k][:])
                    elif bi == nblk_k - 1:
                        lo = h * 128 if h < nblk_k else 0
                        nc.sync.dma_start(
                            outs[k][lo:]
                            .rearrange("(j p) n -> p j n", p=128),
                            o_seg[k][:, :nblk_k - (lo // 128)])

            if loop:
                with tc.For_i(0, loop, 1):
                    body()
            else:
                for _ in range(repeat):
                    body()
    nc.compile()
    return nc


def _get_nc(caps, repeat=1, loop=None):
    key = (tuple(caps), repeat, loop)
    if key not in _nc_cache:
        _nc_cache[key] = _build(caps, repeat=repeat, loop=loop)
    return _nc_cache[key]


def _route(out_feat_size):
    """Map out_feat_size values -> expert index (-1 = matches no expert)."""
    ofs = np.asarray(out_feat_size).astype(np.int64).reshape(-1)
    branch = np.full(ofs.shape, -1, dtype=np.int64)
    for k, s in enumerate(OUT_SIZES):
        branch[ofs == s] = k
    return branch


def _plan(branch):
    """Balanced routing plan: per-expert global index lists split evenly
    across cores, shared capacities, and segment layout [3,2,1,0]."""
    idx_all = {k: np.nonzero(branch == k)[0] for k in range(N_EXPERTS)}
    per_core = [int(-(-len(idx_all[k]) // N_CORES)) for k in range(N_EXPERTS)]
    caps = [int(-(-per_core[k] // 128) * 128) for k in range(N_EXPERTS)]
    return idx_all, tuple(caps)


def kernel(x, weight, bias, out_feat_size):
    import ml_dtypes
    from concourse.bass_utils import run_bass_kernel_spmd

    bf16 = ml_dtypes.bfloat16
    x = np.asarray(x, dtype=np.float32)
    weight = np.asarray(weight, dtype=np.float32)
    bias = np.asarray(bias, dtype=np.float32)
    B, T, D = x.shape
    assert D == IN_FEAT
    n_tok = B * T

    branch = _route(out_feat_size)
    idx_all, caps = _plan(branch)
    if sum(caps) == 0:
        return np.zeros((B, T, IN_FEAT), dtype=np.float32)

    # host-side weight/bias layout
    wt = np.empty((IN_FEAT, W_COLS), dtype=bf16)
    bb = np.empty((W_COLS,), dtype=np.float32)
    for k, ok in enumerate(OUT_SIZES):
        wt[:, WOFF[k]:WOFF[k] + ok] = weight[k, :ok, :].T.astype(bf16)
        bb[WOFF[k]:WOFF[k] + ok] = bias[k, :ok]
    bb128 = np.ascontiguousarray(np.broadcast_to(bb.astype(bf16),
                                                 (128, W_COLS)))
    bbr = np.concatenate([np.tile(bb[WOFF[0]:WOFF[0] + 128], 4),
                          np.tile(bb[WOFF[1]:WOFF[1] + 256], 2)])
    bbr128 = np.ascontiguousarray(np.broadcast_to(bbr.astype(bf16),
                                                  (128, 1024)))

    x2 = x.reshape(n_tok, IN_FEAT).astype(bf16)
    tpad = sum(caps)
    nblocks = -(-tpad // BLK)
    tdma = nblocks * BLK
    seg_off = {}
    t0 = 0
    for k in (3, 2, 1, 0):
        if caps[k]:
            seg_off[k] = t0
            t0 += caps[k]

    in_maps = []
    core_slices = []  # per core: {expert: global idx array}
    for c in range(N_CORES):
        perm = np.zeros(tdma, dtype=np.int64)
        slices = {}
        for k, off in seg_off.items():
            idx = idx_all[k]
            m = int(-(-len(idx) // N_CORES))
            part = idx[c * m:(c + 1) * m]
            slices[k] = part
            if len(part):
                perm[off:off + len(part)] = part
                perm[off + len(part):off + caps[k]] = part[0]
        xtb = np.empty((nblocks, IN_FEAT, BLK), dtype=bf16)
        for ci in range(nblocks):
            np.copyto(xtb[ci], x2[perm[ci * BLK:(ci + 1) * BLK]].T)
        in_maps.append({"xt": xtb, "wt": wt, "bb": bb128, "bbr": bbr128})
        core_slices.append(slices)

    global _LAST_CAPS, _LAST_IN_MAPS
    _LAST_CAPS, _LAST_IN_MAPS = caps, in_maps

    nc = _get_nc(caps)
    res = run_bass_kernel_spmd(nc, in_maps, list(range(N_CORES))).results

    out = np.zeros((n_tok, IN_FEAT), dtype=np.float32)
    for c in range(N_CORES):
        for k, part in core_slices[c].items():
            n = len(part)
            if n == 0:
                continue
            ok = OUT_SIZES[k]
            out[part, :ok] = res[c][f"out{k}"][:n].astype(np.float32)
            if ok < IN_FEAT:
                # reference semantics: bias tail beyond out_k (zero for the
                # shipped inputs, which pre-zero the bias)
                out[part, ok:] = bias[k, ok:]
    return out.reshape(B, T, IN_FEAT)
